# revision 24
# baseline (speedup 1.0000x reference)
"""GQA attention with QK-norm for Trainium2, sharded over 8 NeuronCores.

Problem: B=2, T=2048, D=2048, H=16 query heads, KVH=4 kv heads, dk=128.
    Q = q @ Wq.T ; K = k @ Wk.T ; V = v @ Wv.T  (per batch)
    Q = g * l2norm(Q, per head) ; K = l2norm(K, per head)
    out = softmax(causal(Q K^T / sqrt(dk))) V @ Wo.T

Sharding: core c = 4*b + gi handles batch b and kv-head group gi
(4 query heads + 1 kv head). Each core computes a row-shard of the
output projection (O^T partial over its 512 head-dims); the host sums
the 4 partials per batch. No device collectives.

On-core layout is feature-major ("transposed") throughout; softmax
numerator exp(S^T) needs no max-subtraction because QK-norm bounds
|scores| <= g/sqrt(dk) ~ 0.674.

v2 schedule (PE-density rewrite):
  phase A: K-proj -> K-norm -> Q-proj (tb-outer) -> Q-norms -> V-proj
  -> V-transpose.  Norm chains hide under the next projection's
  matmuls; broadcasts use gpsimd partition_broadcast (no HBM hop).
  phase B: (qb, h) columns are software-pipelined one head ahead:
  PE stream is s1(i), s1(i+1), s2(i), s1(i+2), s2(i+1), ... where
  s1 = scores matmuls (exp on ACT trails one head behind) and
  s2 = rowsum + AV matmuls off the finished exp strip, so the PE
  never waits on the scalar engine.  Diagonal k-tiles are narrowed
  to their live (causal) columns in scores/rowsum/AV/exp.  The
  out-projection of each t-block is emitted two pipeline slots after
  the block's last head, hiding the rowsum-reciprocal chain.
"""

import math
import os
import sys

for _p in ("/opt/trn_rl_repo",):
    if _p not in sys.path:
        sys.path.append(_p)

import numpy as np
from concourse import bacc, mybir, tile
from concourse.bass_utils import run_bass_kernel_spmd
from concourse.masks import make_identity

B, T, D, H, KVH, DK = 2, 2048, 2048, 16, 4, 128
HPG = H // KVH          # query heads per core (group)
E = HPG * DK            # 512: q-head dims per core
P = 128
TB = 4                  # t blocks of 512
NT = T // P             # 16 tiles of 128 along T
ND = D // P             # 16 contraction tiles
f32 = mybir.dt.float32
f32r = mybir.dt.float32r
bf16 = mybir.dt.bfloat16
AF = mybir.ActivationFunctionType
EPS2 = 1e-24

MM_DT = {"f32r": f32r, "bf16": bf16, "f32": f32}[
    os.environ.get("ATTN_DT", "bf16")]
ST_DT = bf16 if MM_DT == bf16 else f32


def _ld(x):
    return x.bitcast(MM_DT) if MM_DT is f32r else x


def build_kernel():
    nc = bacc.Bacc(None, target_bir_lowering=False)

    # host-pre-tiled inputs (see make_in_maps): every DMA below reads
    # contiguous multi-KB rows.
    qTt = nc.declare_dram_parameter("qTt", [TB, P, ND * 512], ST_DT,
                                    isOutput=False)
    kT = nc.declare_dram_parameter("kT", [D, T], ST_DT, isOutput=False)
    vT = nc.declare_dram_parameter("vT", [D, T], ST_DT, isOutput=False)
    wqt = nc.declare_dram_parameter("wqt", [P, ND * E], ST_DT, isOutput=False)
    wkt = nc.declare_dram_parameter("wkt", [P, ND * DK], ST_DT,
                                    isOutput=False)
    wvt = nc.declare_dram_parameter("wvt", [P, ND * DK], ST_DT,
                                    isOutput=False)
    wot = nc.declare_dram_parameter("wot", [P, HPG * D], ST_DT,
                                    isOutput=False)
    gs16 = nc.declare_dram_parameter("gs16", [NT, HPG * P], f32,
                                     isOutput=False)
    outT = nc.declare_dram_parameter("outT", [D, T], bf16, isOutput=True)

    n2_dram = nc.dram_tensor("n2_dram", [HPG + 1, T], f32)
    c_dram = nc.dram_tensor("c_dram", [HPG + 1, T], bf16)

    from contextlib import ExitStack

    with tile.TileContext(nc) as tc:
        with ExitStack() as outer:
            const = outer.enter_context(tc.tile_pool(name="const", bufs=1))
            persist = outer.enter_context(tc.tile_pool(name="persist", bufs=1))

            ident = const.tile([P, P], f32, tag="ident")
            make_identity(nc, ident[:])
            ones_f32 = const.tile([P, 1], f32, tag="ones_f32")
            nc.vector.memset(ones_f32[:], 1.0)
            ones = const.tile([P, 1], MM_DT, tag="ones")
            nc.vector.tensor_copy(ones[:], ones_f32[:])
            gs_sb = const.tile([NT, HPG * P], f32, tag="gs")
            nc.sync.dma_start(gs_sb[:], gs16[:])
            eps16 = const.tile([NT, 1], f32, tag="eps16")
            nc.vector.memset(eps16[:], EPS2)
            # causal keep-mask: M[p, c] = 1.0 iff c >= p + 384; the live
            # part of every diagonal k-tile uses M[:, 384 : 384+W].
            maskM = const.tile([P, 896], f32, tag="mask")
            nc.vector.memset(maskM[:], 1.0)
            nc.gpsimd.affine_select(
                out=maskM[:], in_=maskM[:],
                compare_op=mybir.AluOpType.is_ge,
                fill=0.0, base=-384,
                pattern=[[1, 896]], channel_multiplier=-1,
            )

            qt_sb = persist.tile([P, HPG * T], MM_DT, tag="qt")
            kt_sb = persist.tile([P, T], MM_DT, tag="kt")
            vtm_sb = persist.tile([P, T], MM_DT, tag="vtm")
            yt_sb = persist.tile([P, HPG * T], MM_DT, tag="yt")

            # ---------------- phase A: projections + norms ----------------
            with ExitStack() as pa:
                wpool = pa.enter_context(tc.tile_pool(name="wpool", bufs=1))
                actsq = pa.enter_context(tc.tile_pool(name="actsq", bufs=2))
                actskv = pa.enter_context(tc.tile_pool(name="actskv", bufs=3))
                scratch = pa.enter_context(tc.tile_pool(name="scratch",
                                                        bufs=2))
                bcast = pa.enter_context(tc.tile_pool(name="bcast", bufs=2))
                rows = pa.enter_context(tc.tile_pool(name="rows", bufs=4))
                smal = pa.enter_context(tc.tile_pool(name="smal", bufs=3))
                psA = pa.enter_context(
                    tc.tile_pool(name="psA", bufs=6, space="PSUM"))
                psTP = pa.enter_context(
                    tc.tile_pool(name="psTP", bufs=2, space="PSUM"))

                def proj_kv(src_dram, w_sb, dst_fn):
                    # k/v activations ride the scalar-engine HWDGE queue so
                    # they never queue behind the q loads on the sync queue.
                    accs = [psA.tile([P, 512], f32, tag="proj",
                                     name=f"acc{_t}") for _t in range(TB)]
                    for n in range(ND):
                        a = actskv.tile([P, T], MM_DT, tag="akv")
                        nc.scalar.dma_start(
                            a[:], _ld(src_dram[n * P:(n + 1) * P, :]))
                        for tb in range(TB):
                            nc.tensor.matmul(
                                accs[tb][:],
                                w_sb[:, n * DK:(n + 1) * DK],
                                a[:, tb * 512:(tb + 1) * 512],
                                start=(n == 0), stop=(n == ND - 1))
                    for tb in range(TB):
                        dst_fn(tb, accs[tb])

                def norm_rowsums(xt_tb, idx, tb):
                    """Partition sums of xt_tb^2 [128, 512] via a ones-matmul,
                    staged to n2_dram for the per-head tail."""
                    sq = scratch.tile([P, 512], MM_DT, tag="sqs")
                    nc.vector.tensor_mul(sq[:], xt_tb, xt_tb)
                    ps = psTP.tile([1, 512], f32, tag="tp")
                    nc.tensor.matmul(ps[:], ones[:], sq[:],
                                     start=True, stop=True)
                    n2row = rows.tile([1, 512], f32, tag="n2row")
                    nc.vector.tensor_copy(n2row[:], ps[:])
                    nc.sync.dma_start(
                        n2_dram[idx:idx + 1, tb * 512:(tb + 1) * 512],
                        n2row[:])

                def norm_tail(heads, with_gain):
                    """Fused rsqrt(n2) (* gain) for a list of (idx, xt) in
                    one [16, 128*len] chain, then per-head broadcast-scale."""
                    nh = len(heads)
                    W = P * nh
                    n2c = smal.tile([NT, W], f32, tag="n2c")
                    for j, (idx, _) in enumerate(heads):
                        nc.sync.dma_start(
                            n2c[:, j * P:(j + 1) * P],
                            n2_dram[idx, :].rearrange("(c p) -> c p", p=P))
                    # y = rsqrt(n2) with one Newton step:
                    # y0 = 1/sqrt(n2+eps); y1 = y0*(1.5 - 0.5*n2*y0^2)
                    sq_c = smal.tile([NT, W], f32, tag="sqc")
                    nc.scalar.activation(sq_c[:], n2c[:], AF.Sqrt,
                                         bias=eps16[:])
                    y0 = smal.tile([NT, W], f32, tag="y0")
                    nc.vector.reciprocal_approx_fast(y0[:], sq_c[:])
                    t1 = smal.tile([NT, W], f32, tag="t1")
                    nc.vector.tensor_mul(t1[:], y0[:], y0[:])
                    nc.vector.tensor_mul(t1[:], t1[:], n2c[:])
                    nc.vector.tensor_scalar(
                        out=t1[:], in0=t1[:], scalar1=-0.5, scalar2=1.5,
                        op0=mybir.AluOpType.mult, op1=mybir.AluOpType.add)
                    nc.vector.tensor_mul(y0[:], y0[:], t1[:])
                    if with_gain:
                        nc.vector.tensor_mul(y0[:], y0[:], gs_sb[:, :W])
                    y0c = smal.tile([NT, W], bf16, tag="y0c")
                    nc.vector.tensor_copy(y0c[:], y0[:])
                    for j, (idx, _) in enumerate(heads):
                        nc.sync.dma_start(
                            c_dram[idx, :].rearrange("(c p) -> c p", p=P),
                            y0c[:, j * P:(j + 1) * P])
                    for j, (idx, xt) in enumerate(heads):
                        bc = bcast.tile([P, T], bf16, tag="bc")
                        nc.sync.dma_start(
                            bc[:], c_dram[idx:idx + 1, :].to_broadcast((P, T)))
                        nc.vector.tensor_mul(xt, xt, bc[:])

                def l2normalize(xt, idx, gs_col):
                    for tb in range(TB):
                        norm_rowsums(xt[:, tb * 512:(tb + 1) * 512], idx, tb)
                    norm_tail([(idx, xt)], gs_col is not None)

                # K first: attention needs it earliest, and its norm chain
                # hides under the V/Q projections.
                wk_sb = wpool.tile([P, ND * DK], MM_DT, tag="wk")
                nc.sync.dma_start(wk_sb[:], _ld(wkt[:]))
                proj_kv(kT, wk_sb,
                        lambda tb, ps: nc.any.tensor_copy(
                            kt_sb[:, tb * 512:(tb + 1) * 512], ps[:]))
                l2normalize(kt_sb[:], HPG, None)

                # V second: its chunks stream on the scalar queue while the
                # q blocks load on the sync queue for the Q projection.
                wv_sb = wpool.tile([P, ND * DK], MM_DT, tag="wv")
                nc.sync.dma_start(wv_sb[:], _ld(wvt[:]))
                vt_stage = scratch.tile([P, T], f32, tag="scr")
                proj_kv(vT, wv_sb,
                        lambda tb, ps: nc.any.tensor_copy(
                            vt_stage[:, tb * 512:(tb + 1) * 512], ps[:]))
                for n in range(NT):
                    tp = psTP.tile([P, P], f32, tag="tp")
                    nc.tensor.transpose(
                        tp[:], vt_stage[:, n * P:(n + 1) * P], ident[:])
                    nc.vector.tensor_copy(vtm_sb[:, n * P:(n + 1) * P], tp[:])

                # Q projection, tb-outer, qa streamed in half blocks (8
                # contraction tiles each) so only 2 half-blocks are live.
                wq_sb = wpool.tile([P, ND * E], MM_DT, tag="wq")
                nc.sync.dma_start(wq_sb[:], _ld(wqt[:]))
                NH = ND // 2
                for tb in range(TB):
                    accs = [psA.tile([P, 512], f32, tag="proj",
                                     name=f"qacc{_h}") for _h in range(HPG)]
                    for half in range(2):
                        a = actsq.tile([P, NH * 512], MM_DT, tag="acts")
                        nc.sync.dma_start(
                            a[:], _ld(qTt[tb, :, half * NH * 512:
                                          (half + 1) * NH * 512]))
                        for h in range(HPG):
                            for nn in range(NH):
                                n = half * NH + nn
                                nc.tensor.matmul(
                                    accs[h][:],
                                    wq_sb[:,
                                          n * E + h * P:n * E + (h + 1) * P],
                                    a[:, nn * 512:(nn + 1) * 512],
                                    start=(n == 0), stop=(n == ND - 1))
                    for h in range(HPG):
                        qslice = qt_sb[:, h * T + tb * 512:
                                       h * T + (tb + 1) * 512]
                        nc.vector.tensor_copy(qslice, accs[h][:])
                        norm_rowsums(qslice, h, tb)
                # all 4 heads' rsqrt chains fused into one [16, 512] pass
                norm_tail([(h, qt_sb[:, h * T:(h + 1) * T])
                           for h in range(HPG)], True)

            # ------------- phase B+C: attention + out projection ----------
            atp = outer.enter_context(tc.tile_pool(name="atp", bufs=2))
            bcy = outer.enter_context(tc.tile_pool(name="bcy", bufs=2))
            invp = outer.enter_context(tc.tile_pool(name="invp", bufs=2))
            wo_pool = outer.enter_context(tc.tile_pool(name="wo", bufs=1))
            ostage = outer.enter_context(tc.tile_pool(name="ostage", bufs=3))
            ps_st = outer.enter_context(
                tc.tile_pool(name="ps_st", bufs=3, space="PSUM"))
            ps_y = outer.enter_context(
                tc.tile_pool(name="ps_y", bufs=2, space="PSUM"))
            ps_sums = outer.enter_context(
                tc.tile_pool(name="ps_sums", bufs=2, space="PSUM"))
            ps_o = outer.enter_context(
                tc.tile_pool(name="ps_o", bufs=1, space="PSUM"))

            wo_sb = wo_pool.tile([P, HPG * D], MM_DT, tag="wo")
            nc.sync.dma_start(wo_sb[:], _ld(wot[:]))

            cols = [(qb, h) for qb in range(TB) for h in range(HPG)]
            state = {}   # (qb, h) -> (strip, ps_yt slot deferred)

            def live_w(qb, kt):
                """Live (causal) column count of k-tile kt in q-block qb."""
                j = kt - 4 * qb
                return 512 - 128 * j if j > 0 else 512

            def s1(qb, h):
                """Scores -> exp -> masked strip for one (q-block, head)."""
                n_k = 4 * (qb + 1)
                qh = qt_sb[:, h * T + qb * 512:h * T + (qb + 1) * 512]
                strip = atp.tile([P, NT * 512], MM_DT, tag="strip")
                for kt in range(n_k):
                    w = live_w(qb, kt)
                    c0 = 512 - w
                    st = ps_st.tile([P, 512], f32, tag="st")
                    nc.tensor.matmul(
                        st[:, c0:512],
                        kt_sb[:, kt * P:(kt + 1) * P],
                        qh[:, c0:512], start=True, stop=True)
                    ssl = strip[:, kt * 512 + c0:(kt + 1) * 512]
                    nc.scalar.activation(ssl, st[:, c0:512], AF.Exp)
                    if kt - 4 * qb >= 0:  # diagonal tile: causal mask
                        nc.vector.tensor_mul(
                            ssl, ssl, maskM[:, 384:384 + w])
                state[(qb, h)] = strip

            def s2(qb, h):
                """Rowsum + AV + normalize off the finished strip."""
                n_k = 4 * (qb + 1)
                strip = state.pop((qb, h))
                ps_sm = ps_sums.tile([1, 512], f32, tag="sums")
                for kt in range(n_k):
                    w = live_w(qb, kt)
                    c0 = 512 - w
                    nc.tensor.matmul(
                        ps_sm[:, c0:512], ones[:],
                        strip[:, kt * 512 + c0:(kt + 1) * 512],
                        start=(kt == 0), stop=(kt == n_k - 1))
                inv_row = invp.tile([1, 512], f32, tag="inv")
                nc.vector.reciprocal_approx_fast(inv_row[:], ps_sm[:])
                ps_yt = ps_y.tile([P, 512], f32, tag="y")
                for kt in range(n_k):
                    w = live_w(qb, kt)
                    c0 = 512 - w
                    nc.tensor.matmul(
                        ps_yt[:, c0:512], vtm_sb[:, kt * P:(kt + 1) * P],
                        strip[:, kt * 512 + c0:(kt + 1) * 512],
                        start=(kt == 0), stop=(kt == n_k - 1))
                yslice = yt_sb[:, h * T + qb * 512:h * T + (qb + 1) * 512]
                nc.vector.tensor_copy(yslice, ps_yt[:])
                bc = bcy.tile([P, 512], f32, tag="bcy")
                nc.gpsimd.partition_broadcast(bc[:], inv_row[:])
                nc.vector.tensor_mul(yslice, yslice, bc[:])

            def outproj(tb):
                for ot in range(NT):
                    ps = ps_o.tile([P, 512], f32, tag="o")
                    for h in range(HPG):
                        nc.tensor.matmul(
                            ps[:],
                            wo_sb[:, h * D + ot * P:h * D + (ot + 1) * P],
                            yt_sb[:, h * T + tb * 512:h * T + (tb + 1) * 512],
                            start=(h == 0), stop=(h == HPG - 1))
                    o_sb = ostage.tile([P, 512], bf16, tag="osb")
                    nc.any.tensor_copy(o_sb[:], ps[:])
                    nc.sync.dma_start(
                        outT[ot * P:(ot + 1) * P, tb * 512:(tb + 1) * 512],
                        o_sb[:])

            # software pipeline: s1 runs one column ahead of s2; each
            # t-block's out-projection lands two slots after its last s2.
            pending_out = None
            for i in range(len(cols) + 1):
                if i < len(cols):
                    s1(*cols[i])
                if i >= 1:
                    qb_d, h_d = cols[i - 1]
                    s2(qb_d, h_d)
                    if pending_out is not None:
                        outproj(pending_out)
                        pending_out = None
                    if h_d == HPG - 1:
                        pending_out = qb_d
            if pending_out is not None:
                outproj(pending_out)

    nc.compile()
    return nc


def make_in_maps(q, k, v, Wq, Wk, Wv, Wo, g):
    import ml_dtypes
    st = ml_dtypes.bfloat16 if ST_DT == bf16 else np.float32
    in_maps = []
    act_t = {}
    for b in range(B):
        qTb = np.ascontiguousarray(q[b].T).astype(st)
        # [TB, P, ND*512]: row p of block tb = concat_n qT[n*128+p, tb*512:]
        qTt = np.ascontiguousarray(
            qTb.reshape(ND, P, TB, 512).transpose(2, 1, 0, 3)
            .reshape(TB, P, ND * 512))
        act_t[b] = (
            qTt,
            np.ascontiguousarray(k[b].T).astype(st),
            np.ascontiguousarray(v[b].T).astype(st),
        )

    def wtile(wT, cols):  # wT: (D, cols) -> [P, ND*cols] row-tiled
        return np.ascontiguousarray(
            np.ascontiguousarray(wT).reshape(-1, P, cols)
            .transpose(1, 0, 2).reshape(P, -1)).astype(st)

    g_flat = np.asarray(g, dtype=np.float32).reshape(H)
    for c in range(8):
        b, gi = divmod(c, KVH)
        qTt, kTb, vTb = act_t[b]
        e0 = gi * E
        gvals = g_flat[gi * HPG:(gi + 1) * HPG] / math.sqrt(DK)
        gs_wide = np.repeat(gvals, P)  # [HPG*P], per-head gain replicated
        in_maps.append({
            "qTt": qTt, "kT": kTb, "vT": vTb,
            "wqt": wtile(Wq[e0:e0 + E, :].T, E),
            "wkt": wtile(Wk[gi * DK:(gi + 1) * DK, :].T, DK),
            "wvt": wtile(Wv[gi * DK:(gi + 1) * DK, :].T, DK),
            "wot": wtile(Wo[:, e0:e0 + E].T, D),
            "gs16": np.broadcast_to(gs_wide[None, :], (NT, HPG * P)).copy(),
        })
    return in_maps


_cached = {}


def kernel(q, k, v, Wq, Wk, Wv, Wo, g, _trace=False, _tmpdir=None):
    if "nc" not in _cached:
        _cached["nc"] = build_kernel()
    nc = _cached["nc"]
    in_maps = make_in_maps(
        np.asarray(q, np.float32), np.asarray(k, np.float32),
        np.asarray(v, np.float32), np.asarray(Wq, np.float32),
        np.asarray(Wk, np.float32), np.asarray(Wv, np.float32),
        np.asarray(Wo, np.float32), g)
    res = run_bass_kernel_spmd(
        nc, in_maps, list(range(8)), trace=_trace, tmpdir=_tmpdir)
    out = np.empty((B, T, D), dtype=np.float32)
    for b in range(B):
        acc = res.results[4 * b]["outT"].astype(np.float32)
        for gi in range(1, KVH):
            acc += res.results[4 * b + gi]["outT"].astype(np.float32)
        out[b] = acc.T
    kernel.last_results = res
    return out
